# revision 29
# baseline (speedup 1.0000x reference)
"""Distributed Trainium2 Bass kernel for nn_Attention_11347303596474.

Cross-attention: out = (softmax(LN(latents)Wq (LN(x)Wk)^T / sqrt(dh)) (LN(x)Wv)) Wo + bo
Shapes: x [4,4096,1024], latents [4,512,1024], 8 heads x 64, INNER=512.

Sharding over 8 NeuronCores: core c handles batch b = c//2 and head-half
hh = c%2 (4 heads = 256 inner columns). Host sums the two head-half
partial outputs per batch.

v3 design (vs the 200us baseline):
- x is uploaded BOTH natural [N,DIM] (for per-token stats) and
  pre-transposed [DIM,N] (host transpose), so K/V projections read x^T
  directly and the 256 PE transposes + their Act evictions disappear.
- LayerNorm is folded algebraically instead of applied:
    k_true[n] = r_n (P'k[n] - mu_n s_k)        (bias bk dropped: softmax
                                                shift-invariant per m)
    q_fin[m]  = SCALE r_m (P'q[m] - mu_m s_q) + SCALE bq
    v_true[n] = r_n P'v[n] - r_n mu_n s_v + bv
  where P' = x @ (g-folded W), s = colsums(W'). Mean-centering enters the
  projection PSUM via rank-1 K=1 matmuls (lhsT = s-row, rhs = -mu-row).
  K's r_n is applied as the per-partition `scale` AP of the softmax exp;
  V rows are pre-scaled by r_n during PSUM eviction (DVE tensor_scalar).
  V's mean term rides as a 66th V column (mu*r) through the AV matmul and
  is removed by a rank-1 term in the output projection (w2 rows).
- Attention is fused into the x-stream: sim lags projections by 1 block,
  AV(i=0 head pair) by 2 blocks, so the ~66us of Act exp hides under PE
  work and the PE stays continuously busy (pstate ramp => 2.4 GHz).
- The i=1 head pair's et tiles persist in SBUF (64KB/part) and its AV runs
  as a short pure-PE phase B (PSUM budget: avp4 banks would not fit
  alongside sim4 + proj2 in phase A).
- Per-token stats on DVE via scalar_tensor_tensor accum passes
  (Sum(x): (x*1) max x; Sum(x^2): (x*1)*x); tensor_tensor_reduce
  crashes the HW DVE exec unit, do not use it.
"""
import os
import numpy as np
import ml_dtypes

import concourse.bacc as bacc
import concourse.mybir as mybir
import concourse.tile as tile
from concourse import bass_utils, masks
from contextlib import ExitStack

f32 = mybir.dt.float32
bf16 = mybir.dt.bfloat16
AF = mybir.ActivationFunctionType
ALU = mybir.AluOpType

B, N, M, DIM = 4, 4096, 512, 1024
HEADS, DH = 8, 64
INNER = HEADS * DH
SCALE = DH ** -0.5
JC = 256          # inner columns per core (4 heads)
NB = N // 512     # 8 token blocks of 512
EPS = 1e-5

_CACHE = {}
KPHASE = int(os.environ.get("BASS_KPHASE", "5"))
# 1=proj only, 2=+sim/exp, 3=+AV_i0, 4=+phaseB, 5=full


def _build():
    nc = bacc.Bacc("TRN2", target_bir_lowering=False, debug=False)

    x_d = nc.declare_dram_parameter("x", [N, DIM], bf16, isOutput=False)
    xT_d = nc.declare_dram_parameter("xT", [DIM, N], bf16, isOutput=False)
    lat_d = nc.declare_dram_parameter("lat", [M, DIM], bf16, isOutput=False)
    latT_d = nc.declare_dram_parameter("latT", [DIM, M], bf16, isOutput=False)
    wq_d = nc.declare_dram_parameter("wq", [DIM, JC], bf16, isOutput=False)
    wk_d = nc.declare_dram_parameter("wk", [DIM, JC], bf16, isOutput=False)
    wv_d = nc.declare_dram_parameter("wv", [DIM, JC], bf16, isOutput=False)
    sk_d = nc.declare_dram_parameter("sk", [1, JC], bf16, isOutput=False)
    sq_d = nc.declare_dram_parameter("sq", [1, JC], bf16, isOutput=False)
    sv_d = nc.declare_dram_parameter("sv", [1, JC], bf16, isOutput=False)
    bqs_d = nc.declare_dram_parameter("bqs", [2, 128], f32, isOutput=False)
    wo_d = nc.declare_dram_parameter("wo", [JC, DIM], bf16, isOutput=False)
    wx_d = nc.declare_dram_parameter("wx", [4, DIM], bf16, isOutput=False)
    wc_d = nc.declare_dram_parameter("wconst", [1, DIM], bf16, isOutput=False)
    out_d = nc.declare_dram_parameter("out", [M, DIM], f32, isOutput=True)

    with tile.TileContext(nc) as tc, ExitStack() as ctx:
        cpool = ctx.enter_context(tc.tile_pool(name="consts", bufs=1))
        wpool = ctx.enter_context(tc.tile_pool(name="weights", bufs=1))
        big = ctx.enter_context(tc.tile_pool(name="big", bufs=1))

        # constants
        ident_f = cpool.tile([128, 128], f32)
        masks.make_identity(nc, ident_f[:])
        ident = cpool.tile([128, 128], bf16)
        nc.vector.tensor_copy(ident[:], ident_f[:])
        eps_t = cpool.tile([128, 1], f32)
        nc.gpsimd.memset(eps_t[:], EPS)

        # weight tiles
        wq = wpool.tile([128, 8 * JC], bf16)
        wk = wpool.tile([128, 8 * JC], bf16)
        wv = wpool.tile([128, 8 * JC], bf16)
        wqv = wq[:].rearrange("p (j i) -> p j i", j=8)
        wkv = wk[:].rearrange("p (j i) -> p j i", j=8)
        wvv = wv[:].rearrange("p (j i) -> p j i", j=8)
        wo = wpool.tile([128, 2 * DIM], bf16)
        wov = wo[:].rearrange("p (i n) -> p i n", i=2)
        wx = wpool.tile([4, DIM], bf16)
        wconst = wpool.tile([1, DIM], bf16)
        sk = wpool.tile([1, JC], bf16)
        sq = wpool.tile([1, JC], bf16)
        sv = wpool.tile([1, JC], bf16)
        bqs = wpool.tile([128, 2], f32)

        # persistent activations
        qT = big.tile([128, 2 * M], bf16)            # Q^T [i][128, 512]
        qTv = qT[:].rearrange("p (i m) -> p i m", i=2)
        kT = big.tile([128, 2 * N], bf16)            # K^T [i][128, 4096]
        kTv = kT[:].rearrange("p (i n) -> p i n", i=2)
        vsb = big.tile([128, 32 * 4 * 66], bf16)     # V token-major (+1, mu*r)
        vv = vsb[:].rearrange("p (c h e) -> p c h e", c=32, h=4)
        et1 = big.tile([128, 32 * 1024], bf16)       # persisted exp for i=1
        et1v = et1[:].rearrange("p (c m) -> p c m", c=32)
        avs = big.tile([128, 4 * M], bf16)           # evicted AV + (D, W) rows
        avsv = avs[:].rearrange("p (h m) -> p h m", h=4)
        aot = big.tile([128, 2 * M], bf16)           # normalized attn_out^T
        aotv = aot[:].rearrange("p (i m) -> p i m", i=2)
        gbuf = big.tile([4, M], bf16)                # G rows (per m)
        gcols = big.tile([128, 16], f32)             # G per (mt, h) columns
        murow = big.tile([1, NB * 4 * 128], bf16)    # -mu rows per (blk, chunk)
        murv = murow[:].rearrange("p (b c n) -> p b c n", b=NB, c=4)
        murf = murow[:].rearrange("p (b n) -> p b n", b=NB)
        mulr = big.tile([1, 4 * 128], bf16)          # -mu rows for latents
        mulrv = mulr[:].rearrange("p (t n) -> p t n", t=4)

        # per-token stats, packed [128, 32] (column = chunk)
        sx = big.tile([128, 32], f32)
        ssq = big.tile([128, 32], f32)
        mu = big.tile([128, 32], f32)
        musq = big.tile([128, 32], f32)
        vvar = big.tile([128, 32], f32)
        sd = big.tile([128, 32], f32)
        rr = big.tile([128, 32], f32)
        rmu = big.tile([128, 32], f32)
        nmu = big.tile([128, 32], f32)
        # latent stats
        lsx = big.tile([128, 4], f32)
        lssq = big.tile([128, 4], f32)
        lmu = big.tile([128, 4], f32)
        lmusq = big.tile([128, 4], f32)
        lvar = big.tile([128, 4], f32)
        lsd = big.tile([128, 4], f32)
        lrt = big.tile([128, 4], f32)
        lrs = big.tile([128, 4], f32)
        lnmu = big.tile([128, 4], f32)

        ones_f = cpool.tile([1, 128], f32)
        nc.gpsimd.memset(ones_f[:], 1.0)
        ones_row = cpool.tile([1, 128], bf16)
        nc.vector.tensor_copy(ones_row[:], ones_f[:])
        # vv col 64 = 1.0 for all chunks/heads (D denominator rides AV)
        ones128 = cpool.tile([128, 128], f32)
        nc.gpsimd.memset(ones128[:], 1.0)
        nc.vector.tensor_copy(
            vv[:, :, :, 64:65],
            ones128[:].rearrange("p (c h u) -> p c h u", c=32, h=4))

        # streaming pools (closed before phase C to free SBUF)
        sctx = ExitStack()
        xnat = sctx.enter_context(tc.tile_pool(name="xnat", bufs=6))
        xTp = sctx.enter_context(tc.tile_pool(name="xT", bufs=2))
        latp = sctx.enter_context(tc.tile_pool(name="latp", bufs=4))
        latTp = sctx.enter_context(tc.tile_pool(name="latTp", bufs=1))
        et0p = sctx.enter_context(tc.tile_pool(name="et0", bufs=9))
        scr = sctx.enter_context(tc.tile_pool(name="scratch", bufs=1))
        gscr = sctx.enter_context(tc.tile_pool(name="gscratch", bufs=1))

        def stats_block(xts, base, sx_, ssq_, mu_, musq_, var_, sd_, r_,
                        rmu_, nmu_, nchunks=4):
            """Per-token LN stats for `nchunks` [128,1024] natural tiles."""
            for c in range(nchunks):
                xt = xts[c]
                g1 = gscr.tile([128, DIM], bf16, name="gs")
                nc.vector.scalar_tensor_tensor(
                    g1[:], xt[:], 1.0, xt[:], ALU.mult, ALU.max,
                    accum_out=sx_[:, base + c:base + c + 1])
                s1 = scr.tile([128, DIM], bf16, name="sq")
                nc.vector.scalar_tensor_tensor(
                    s1[:], xt[:], 1.0, xt[:], ALU.mult, ALU.mult,
                    accum_out=ssq_[:, base + c:base + c + 1])
            sl = slice(base, base + nchunks)
            nc.vector.tensor_scalar(mu_[:, sl], sx_[:, sl], 1.0 / DIM, None,
                                    ALU.mult)
            # var = ssq/DIM - mu^2  (E[x^2] - mu^2)
            nc.vector.tensor_tensor(musq_[:, sl], mu_[:, sl], mu_[:, sl],
                                    ALU.mult)
            nc.vector.scalar_tensor_tensor(var_[:, sl], ssq_[:, sl], 1.0 / DIM,
                                           musq_[:, sl], ALU.mult,
                                           ALU.subtract)
            nc.scalar.activation(sd_[:, sl], var_[:, sl], AF.Sqrt,
                                 bias=eps_t[:])
            nc.vector.reciprocal(r_[:, sl], sd_[:, sl])
            if rmu_ is not None:
                nc.vector.tensor_tensor(rmu_[:, sl], r_[:, sl], mu_[:, sl],
                                        ALU.mult)
            nc.vector.tensor_scalar(nmu_[:, sl], mu_[:, sl], -1.0, None,
                                    ALU.mult)

        with tc.tile_pool(name="av_ps", bufs=1, space="PSUM") as av_ps:
            avp0 = av_ps.tile([128, M], f32, tag="avp0")
            avp1 = av_ps.tile([128, M], f32, tag="avp1")

            with tc.tile_pool(name="proj_ps", bufs=2, space="PSUM") as proj_ps, \
                 tc.tile_pool(name="sim_ps", bufs=2, space="PSUM") as sim_ps:

                # ---- DMA: latents + first x blocks first, then weights ----
                lat_tiles = []
                for t in range(4):
                    xt = latp.tile([128, DIM], bf16, name="lat")
                    nc.sync.dma_start(xt[:], lat_d[t * 128:(t + 1) * 128, :])
                    lat_tiles.append(xt)
                latTt = latTp.tile([128, 8 * M], bf16, name="latT")
                latTv = latTt[:].rearrange("p (j m) -> p j m", j=8)
                nc.sync.dma_start(
                    latTv, latT_d.ap().rearrange("(j p) m -> p j m", p=128))

                xnat_tiles = {}
                xT_tiles = {}

                def dma_block(b):
                    tl = []
                    for t in range(4):
                        tok = b * 4 + t
                        xt = xnat.tile([128, DIM], bf16, name="xn")
                        nc.sync.dma_start(xt[:],
                                          x_d[tok * 128:(tok + 1) * 128, :])
                        tl.append(xt)
                    xnat_tiles[b] = tl
                    xtt = xTp.tile([128, 8 * 512], bf16, name="xtt")
                    xnv = xtt[:].rearrange("p (j n) -> p j n", j=8)
                    nc.sync.dma_start(
                        xnv, xT_d[:, b * 512:(b + 1) * 512].rearrange(
                            "(j p) n -> p j n", p=128))
                    xT_tiles[b] = xnv

                dma_block(0)

                nc.sync.dma_start(wqv, wq_d.ap().rearrange("(j p) i -> p j i",
                                                           p=128))
                nc.sync.dma_start(wkv, wk_d.ap().rearrange("(j p) i -> p j i",
                                                           p=128))
                nc.sync.dma_start(wvv, wv_d.ap().rearrange("(j p) i -> p j i",
                                                           p=128))
                nc.sync.dma_start(sk[:], sk_d[:, :])
                nc.sync.dma_start(sq[:], sq_d[:, :])
                nc.sync.dma_start(sv[:], sv_d[:, :])
                nc.sync.dma_start(bqs[:],
                                  bqs_d.ap().rearrange("i p -> p i"))
                nc.sync.dma_start(wov, wo_d.ap().rearrange("(i p) n -> p i n",
                                                           p=128))
                nc.sync.dma_start(wx[:], wx_d[:, :])
                nc.sync.dma_start(wconst[:], wc_d[:, :])

                # ---- latent stats + -mu row ----
                stats_block(lat_tiles, 0, lsx, lssq, lmu, lmusq, lvar, lsd,
                            lrt, None, lnmu)
                # lrt holds 1/sd; lrs = SCALE/sd (q scale)
                nc.vector.tensor_scalar(lrs[:], lrt[:], SCALE, None, ALU.mult)
                for t in range(4):
                    tpt = proj_ps.tile([128, 512], f32, name="pp")
                    tp = tpt[0:1, 0:128]
                    nc.tensor.matmul(tp, lnmu[:, t:t + 1], ident_f[:],
                                     is_transpose=True)
                    nc.vector.tensor_copy(mulrv[:, t, :], tp)

                # ---- Q projection (token-major) + fixup + transpose ----
                for mt in range(4):
                    pst = proj_ps.tile([128, 512], f32, name="pp")
                    ps = pst[:, 0:JC]
                    for j in range(8):
                        nc.tensor.matmul(ps,
                                         latTv[:, j, mt * 128:(mt + 1) * 128],
                                         wqv[:, j, :], start=(j == 0),
                                         stop=False)
                    nc.tensor.matmul(ps, mulrv[:, mt, :], sq[:, :],
                                     start=False, stop=True)
                    q2 = scr.tile([128, JC], f32, name="q2")
                    nc.vector.tensor_scalar(q2[:], ps,
                                            lrs[:, mt:mt + 1], None, ALU.mult)
                    for i in range(2):
                        ptt = proj_ps.tile([128, 512], f32, name="pp")
                        pt = ptt[:, 0:128]
                        nc.tensor.matmul(pt, q2[:, i * 128:(i + 1) * 128],
                                         ident_f[:], is_transpose=True)
                        nc.scalar.activation(
                            qTv[:, i, mt * 128:(mt + 1) * 128], pt,
                            AF.Identity, bias=bqs[:, i:i + 1])

                # ---- fused block loop ----
                def stats_emit(b):
                    """DVE/GpSimd/Act stats work for block b (no PE)."""
                    stats_block(xnat_tiles[b], 4 * b, sx, ssq, mu, musq,
                                vvar, sd, rr, rmu, nmu)
                    # vv col 65 = r*mu per chunk (V mean-correction column)
                    for h in range(4):
                        nc.vector.tensor_copy(
                            vv[:, 4 * b:4 * b + 4, h, 65:66],
                            rmu[:, 4 * b:4 * b + 4].rearrange(
                                "p (c u) -> p c u", u=1))

                def murow_T(b):
                    """PE transpose of -mu columns into murow rows (block b)."""
                    for c in range(4):
                        tpt = proj_ps.tile([128, 512], f32, name="pp")
                        tp = tpt[0:1, 0:128]
                        nc.tensor.matmul(tp, nmu[:, 4 * b + c:4 * b + c + 1],
                                         ident_f[:], is_transpose=True)
                        nc.vector.tensor_copy(murv[:, b, c, :], tp)

                def kproj(b, i):
                    ps = proj_ps.tile([128, 512], f32, name="pp")[:]
                    for j in range(8):
                        nc.tensor.matmul(ps,
                                         wkv[:, j, i * 128:(i + 1) * 128],
                                         xT_tiles[b][:, j, :],
                                         start=(j == 0), stop=False)
                    nc.tensor.matmul(ps, sk[:, i * 128:(i + 1) * 128],
                                     murf[:, b, :], start=False, stop=True)
                    nc.vector.tensor_copy(kTv[:, i, b * 512:(b + 1) * 512],
                                          ps)

                def vproj(b, t):
                    ps = proj_ps.tile([128, 512], f32, name="pp")[:, 0:JC]
                    for j in range(8):
                        nc.tensor.matmul(ps,
                                         xT_tiles[b][:, j,
                                                     t * 128:(t + 1) * 128],
                                         wvv[:, j, :], start=(j == 0),
                                         stop=False)
                    nc.tensor.matmul(ps, murv[:, b, t, :], sv[:, :],
                                     start=False, stop=True)
                    c = b * 4 + t
                    nc.vector.tensor_scalar(
                        vv[:, c, :, 0:64],
                        ps.rearrange("p (h e) -> p h e", h=4),
                        rr[:, c:c + 1], None, ALU.mult)

                def sim_chunk(b, i, c):
                    """sim + exp for chunk c of block b, head pair i."""
                    ch = b * 4 + c
                    sp = sim_ps.tile([128, 1024], f32, name="sp")
                    nc.tensor.matmul(sp[:, 0:512],
                                     kTv[0:64, i, ch * 128:(ch + 1) * 128],
                                     qTv[0:64, i, :], start=True, stop=True,
                                     tile_position=(0, 0))
                    nc.tensor.matmul(sp[:, 512:1024],
                                     kTv[64:128, i, ch * 128:(ch + 1) * 128],
                                     qTv[64:128, i, :], start=True, stop=True,
                                     tile_position=(64, 0))
                    if i == 0:
                        et = et0p.tile([128, 1024], bf16, name="et")
                        et0_tiles[ch] = et
                        dst = et[:]
                    else:
                        dst = et1v[:, ch, :]
                    nc.scalar.activation(dst, sp[:], AF.Exp,
                                         scale=rr[:, ch:ch + 1])

                def av0_chunk(b, c):
                    ch = b * 4 + c
                    et = et0_tiles.pop(ch)
                    nc.tensor.matmul(avp0[0:66, :], vv[:, ch, 0, :],
                                     et[:, 0:512], start=(ch == 0),
                                     stop=(ch == 31), skip_group_check=True)
                    nc.tensor.matmul(avp1[0:66, :], vv[:, ch, 1, :],
                                     et[:, 512:1024], start=(ch == 0),
                                     stop=(ch == 31), skip_group_check=True)

                et0_tiles = {}
                stats_emit(0)
                murow_T(0)
                for b in range(NB):
                    if b + 1 < NB:
                        dma_block(b + 1)
                    # interleave: proj(b) | stats(b+1) | sim(b-1) | av0(b-2)
                    if b + 1 < NB:
                        stats_emit(b + 1)
                    kproj(b, 0)
                    if b >= 1 and KPHASE >= 2:
                        sim_chunk(b - 1, 0, 0)
                        sim_chunk(b - 1, 0, 1)
                    kproj(b, 1)
                    if b >= 1 and KPHASE >= 2:
                        sim_chunk(b - 1, 0, 2)
                        sim_chunk(b - 1, 0, 3)
                    vproj(b, 0)
                    vproj(b, 1)
                    if b >= 1 and KPHASE >= 2:
                        sim_chunk(b - 1, 1, 0)
                        sim_chunk(b - 1, 1, 1)
                    vproj(b, 2)
                    vproj(b, 3)
                    if b >= 1 and KPHASE >= 2:
                        sim_chunk(b - 1, 1, 2)
                        sim_chunk(b - 1, 1, 3)
                    if b >= 2 and KPHASE >= 3:
                        for c in range(4):
                            av0_chunk(b - 2, c)
                    if b + 1 < NB:
                        murow_T(b + 1)
                    del xnat_tiles[b]
                    del xT_tiles[b]

                # drain: sim(7), av0(6), av0(7)
                if KPHASE >= 2:
                    for i in range(2):
                        for c in range(4):
                            sim_chunk(NB - 1, i, c)
                if KPHASE >= 3:
                    for c in range(4):
                        av0_chunk(NB - 2, c)
                    for c in range(4):
                        av0_chunk(NB - 1, c)

            # ---- phase B: AV(i=1) + fixup + G ----
            if KPHASE >= 4:
              with tc.tile_pool(name="f1_ps", bufs=2, space="PSUM") as f1_ps, \
                 tc.tile_pool(name="f2_ps", bufs=2, space="PSUM") as f2_ps, \
                 tc.tile_pool(name="fix_sb", bufs=8) as fix_sb:

                # evict i=0 accumulators, then reuse banks for i=1
                nc.vector.tensor_copy(avsv[0:66, 0, :], avp0[0:66, :])
                nc.vector.tensor_copy(avsv[0:66, 1, :], avp1[0:66, :])
                avq0 = av_ps.tile([128, M], f32, tag="avp0")
                avq1 = av_ps.tile([128, M], f32, tag="avp1")
                for ch in range(32):
                    nc.tensor.matmul(avq0[0:66, :], vv[:, ch, 2, :],
                                     et1v[:, ch, 0:512], start=(ch == 0),
                                     stop=(ch == 31), skip_group_check=True)
                    nc.tensor.matmul(avq1[0:66, :], vv[:, ch, 3, :],
                                     et1v[:, ch, 512:1024], start=(ch == 0),
                                     stop=(ch == 31), skip_group_check=True)
                nc.vector.tensor_copy(avsv[0:66, 2, :], avq0[0:66, :])
                nc.vector.tensor_copy(avsv[0:66, 3, :], avq1[0:66, :])

                def fixup(h, mt):
                    """avs[h] -> aot rows; G into gcols."""
                    f1 = f1_ps.tile([128, 128], bf16, name="f1")
                    nc.tensor.matmul(f1[:, 0:66],
                                     avsv[0:66, h, mt * 128:(mt + 1) * 128],
                                     ident[0:66, 0:66], is_transpose=True)
                    rec = fix_sb.tile([128, 1], f32, name="rec")
                    nc.vector.reciprocal(rec[:], f1[:, 64:65])
                    at = fix_sb.tile([128, 64], bf16, name="at")
                    nc.vector.tensor_scalar(at[:], f1[:, 0:64], rec[:],
                                            None, ALU.mult)
                    nc.vector.tensor_scalar(gcols[:, mt * 4 + h:mt * 4 + h + 1],
                                            f1[:, 65:66], rec[:], None,
                                            ALU.mult)
                    f2 = f2_ps.tile([128, 128], bf16, name="f2")
                    nc.tensor.matmul(f2[0:64, :], at[:], ident[:],
                                     is_transpose=True)
                    i, r0 = h // 2, (h % 2) * 64
                    nc.vector.tensor_copy(
                        aotv[r0:r0 + 64, i, mt * 128:(mt + 1) * 128],
                        f2[0:64, :])

                for mt in range(4):
                    for h in range(4):
                        fixup(h, mt)
                gv = gcols[:].rearrange("p (t h) -> p t h", t=4)
                for mt in range(4):
                    gp = f1_ps.tile([4, 128], f32, name="gT")
                    nc.tensor.matmul(gp[:], gv[:, mt, :], ident_f[:],
                                     is_transpose=True)
                    nc.vector.tensor_copy(
                        gbuf[:, mt * 128:(mt + 1) * 128], gp[:])

        sctx.close()

        # ---- phase C: output projection ----
        if KPHASE < 5:
            with tc.tile_pool(name="dummy", bufs=1) as dpool:
                dz = dpool.tile([128, DIM], f32)
                nc.gpsimd.memset(dz[:], 0.0)
                for mt in range(4):
                    nc.sync.dma_start(out_d[mt * 128:(mt + 1) * 128, :], dz[:])
        else:
          with tc.tile_pool(name="o_ps", bufs=2, space="PSUM") as o_ps, \
             tc.tile_pool(name="o_sb", bufs=2) as o_sb:
            for mt in range(4):
                ps = o_ps.tile([128, DIM], f32)
                for ncol in range(2):
                    sl = ps[:, ncol * 512:(ncol + 1) * 512]
                    nc.tensor.matmul(sl, aotv[:, 0, mt * 128:(mt + 1) * 128],
                                     wov[:, 0, ncol * 512:(ncol + 1) * 512],
                                     start=True, stop=False,
                                     skip_group_check=True)
                    nc.tensor.matmul(sl, aotv[:, 1, mt * 128:(mt + 1) * 128],
                                     wov[:, 1, ncol * 512:(ncol + 1) * 512],
                                     start=False, stop=False,
                                     skip_group_check=True)
                    nc.tensor.matmul(sl, gbuf[:, mt * 128:(mt + 1) * 128],
                                     wx[:, ncol * 512:(ncol + 1) * 512],
                                     start=False, stop=False,
                                     skip_group_check=True)
                    nc.tensor.matmul(sl, ones_row[:, :],
                                     wconst[:, ncol * 512:(ncol + 1) * 512],
                                     start=False, stop=True,
                                     skip_group_check=True)
                ot = o_sb.tile([128, DIM], f32)
                nc.scalar.copy(ot[:], ps[:])
                nc.sync.dma_start(out_d[mt * 128:(mt + 1) * 128, :], ot[:])

    nc.compile()
    return nc


def _get_nc():
    if "nc" not in _CACHE:
        _CACHE["nc"] = _build()
    return _CACHE["nc"]


def kernel(x, latents, Wq, Wk, Wv, Wo, bo, gx, bx, gl, bl):
    x = np.asarray(x, dtype=np.float32)
    latents = np.asarray(latents, dtype=np.float32)
    Wq = np.asarray(Wq, np.float32); Wk = np.asarray(Wk, np.float32)
    Wv = np.asarray(Wv, np.float32); Wo = np.asarray(Wo, np.float32)
    bo = np.asarray(bo, np.float32)
    gx = np.asarray(gx, np.float32); bx = np.asarray(bx, np.float32)
    gl = np.asarray(gl, np.float32); bl = np.asarray(bl, np.float32)

    # fold LN affine params into projection weights (host-side, cheap)
    Wqs = gl[:, None] * Wq            # NO SCALE here (applied via lrs)
    bq_full = (bl @ Wq) * SCALE       # [INNER]
    Wks = gx[:, None] * Wk
    Wvs = gx[:, None] * Wv
    bv_full = bx @ Wv

    sq_full = Wqs.sum(0)              # [INNER]
    sk_full = Wks.sum(0)
    sv_full = Wvs.sum(0)

    bf = ml_dtypes.bfloat16
    nc = _get_nc()
    xb = x.astype(bf)
    lb = latents.astype(bf)
    in_maps = []
    for c in range(8):
        b, hh = c // 2, c % 2
        J = slice(hh * JC, (hh + 1) * JC)
        WoJ = Wo[J, :]                              # [256, 1024]
        wconst = bv_full[J] @ WoJ
        if hh == 0:
            wconst = wconst + bo
        svJ = sv_full[J]
        w2 = np.stack([-(svJ[h * 64:(h + 1) * 64] @
                         WoJ[h * 64:(h + 1) * 64, :]) for h in range(4)])

        in_maps.append({
            "x": np.ascontiguousarray(xb[b]),
            "xT": np.ascontiguousarray(xb[b].T),
            "lat": np.ascontiguousarray(lb[b]),
            "latT": np.ascontiguousarray(lb[b].T),
            "wq": np.ascontiguousarray(Wqs[:, J]).astype(bf),
            "wk": np.ascontiguousarray(Wks[:, J]).astype(bf),
            "wv": np.ascontiguousarray(Wvs[:, J]).astype(bf),
            "sk": np.ascontiguousarray(sk_full[J].reshape(1, JC)).astype(bf),
            "sq": np.ascontiguousarray(sq_full[J].reshape(1, JC)).astype(bf),
            "sv": np.ascontiguousarray(svJ.reshape(1, JC)).astype(bf),
            "bqs": np.ascontiguousarray(bq_full[J].reshape(2, 128)),
            "wo": np.ascontiguousarray(WoJ).astype(bf),
            "wx": np.ascontiguousarray(w2).astype(bf),
            "wconst": np.ascontiguousarray(wconst.reshape(1, DIM)).astype(bf),
        })

    res = bass_utils.run_bass_kernel_spmd(nc, in_maps, core_ids=list(range(8)))
    out = np.empty((B, M, DIM), np.float32)
    for b in range(B):
        out[b] = res.results[2 * b]["out"] + res.results[2 * b + 1]["out"]
    return out
